# revision 1
# baseline (speedup 1.0000x reference)
"""Distillation loss (CE + top-k combo KLs + rNTK KL) on 8 Trainium2 cores.

Math: the reference's additive -1000 masks exactly restrict each softmax to
the unmasked entries (exp(-1000-ish) == 0.0 in fp32).  The loss therefore
decomposes into per-row scalars computable from single streaming passes:

  Zce = sum_v exp(s_v)          (CE logsumexp, temp 1)
  Zs4 = sum_v exp(s_v/4)        (student, temp 4)
  Zt4 = sum_v exp(t_v/4)        (teacher, temp 4)
  G   = sum_v exp(t_v/4)*(t_v - s_v)
  top-3 values + indices of s (per row)

Device (data-parallel over the batch, 256 rows/core): streams both logit
matrices once from HBM, producing per-chunk partial sums + top-8-per-chunk
candidates.  Host epilogue (O(B*K) work in float64): exact top-3 from
candidates, teacher gathers, the 3-term correction sums, the 4 tiny combo
KLs, and the final scalar.

Engine split per [128 x 4000] chunk:
  sync  : 2 HBM loads (each split across all 16 SDMA engines)
  DVE   : max8 + max_index          (1-port ops -> dedicated SBUF ports)
  ACT   : exp(t/4), exp(s), exp(s/4) with fused accumulate
  Pool  : 2 fused multiply-reduce STTs (uses the DVE/Pool shared port pair,
          which stays free because DVE never issues a 2-port op)
"""

import sys

import numpy as np

try:
    import concourse.bass as bass
except ImportError:  # pragma: no cover
    sys.path.insert(0, "/opt/trn_rl_repo")
    import concourse.bass as bass

import concourse.bacc as bacc
import concourse.mybir as mybir
from concourse.bass_utils import run_bass_kernel_spmd
from concourse.tile import TileContext

# Problem shape (hardcoded per spec).
B, V = 2048, 32000
NCORES = 8
RPC = B // NCORES          # rows per core = 256
P = 128                    # partitions
NT = RPC // P              # row tiles per core = 2
W = 4000                   # chunk width
NCH = V // W               # chunks per row tile = 8
K = 3
TEMP = 4.0
GAMMA = 0.05

F32 = mybir.dt.float32
U32 = mybir.dt.uint32

_NC = None


def _build_bass():
    global _NC
    if _NC is not None:
        return _NC

    nc = bacc.Bacc("TRN2", target_bir_lowering=False)

    s_d = nc.dram_tensor("student", [RPC, V], F32, kind="ExternalInput")
    t_d = nc.dram_tensor("teacher", [RPC, V], F32, kind="ExternalInput")
    # Per-chunk partials; host reduces. stats_act cols: [Zce | Zs4 | Zt4],
    # stats_g cols: [G] where G = sum(exp(t/4)*(t-s)).
    stats_a_d = nc.dram_tensor("stats_act", [NT, P, 3 * NCH], F32, kind="ExternalOutput")
    stats_p_d = nc.dram_tensor("stats_g", [NT, P, NCH], F32, kind="ExternalOutput")
    cvals_d = nc.dram_tensor("cand_vals", [NT, P, 8 * NCH], F32, kind="ExternalOutput")
    cidx_d = nc.dram_tensor("cand_idx", [NT, P, 8 * NCH], U32, kind="ExternalOutput")

    EXP = mybir.ActivationFunctionType.Exp
    MUL = mybir.AluOpType.mult
    SUB = mybir.AluOpType.subtract
    ADD = mybir.AluOpType.add

    with TileContext(nc) as tc:
        with (
            tc.tile_pool(name="s", bufs=3) as s_pool,
            tc.tile_pool(name="t", bufs=3) as t_pool,
            tc.tile_pool(name="e", bufs=2) as e_pool,
            tc.tile_pool(name="d", bufs=2) as d_pool,
            tc.tile_pool(name="scr", bufs=1) as scr_pool,
            tc.tile_pool(name="small", bufs=2) as small_pool,
        ):
            # Write-only sink for the two student exps (ACT in-order; WAW only).
            scr_act = scr_pool.tile([P, W], F32)

            for t in range(NT):
                sa = small_pool.tile([P, 3 * NCH], F32, tag="sa")
                sp = small_pool.tile([P, NCH], F32, tag="sp")
                cv = small_pool.tile([P, 8 * NCH], F32, tag="cv")
                ci = small_pool.tile([P, 8 * NCH], U32, tag="ci")
                r0 = t * P
                for c in range(NCH):
                    st = s_pool.tile([P, W], F32)
                    tt = t_pool.tile([P, W], F32)
                    et = e_pool.tile([P, W], F32)
                    dt = d_pool.tile([P, W], F32)
                    c0 = c * W
                    nc.sync.dma_start(out=st[:], in_=s_d[r0:r0 + P, c0:c0 + W])
                    nc.sync.dma_start(out=tt[:], in_=t_d[r0:r0 + P, c0:c0 + W])

                    # ACT: exp_t first so DVE's ttr unblocks early.
                    nc.scalar.activation(
                        out=et[:], in_=tt[:], func=EXP, scale=0.25,
                        accum_out=sa[:, 2 * NCH + c:2 * NCH + c + 1],
                    )
                    nc.scalar.activation(
                        out=scr_act[:], in_=st[:], func=EXP, scale=1.0,
                        accum_out=sa[:, c:c + 1],
                    )
                    nc.scalar.activation(
                        out=scr_act[:], in_=st[:], func=EXP, scale=0.25,
                        accum_out=sa[:, NCH + c:NCH + c + 1],
                    )

                    # Pool: diff = t - s (plain 2-input elementwise).
                    nc.gpsimd.tensor_tensor(out=dt[:], in0=tt[:], in1=st[:], op=SUB)

                    # DVE: per-chunk top-8 values + chunk-local indices,
                    # then fused multiply-reduce G_c = sum(diff * exp_t).
                    nc.vector.max(out=cv[:, c * 8:(c + 1) * 8], in_=st[:])
                    nc.vector.max_index(
                        out=ci[:, c * 8:(c + 1) * 8],
                        in_max=cv[:, c * 8:(c + 1) * 8],
                        in_values=st[:],
                    )
                    nc.vector.scalar_tensor_tensor(
                        out=dt[:], in0=dt[:], scalar=1.0, in1=et[:],
                        op0=MUL, op1=MUL,
                        accum_out=sp[:, c:c + 1],
                    )

                nc.sync.dma_start(out=stats_a_d[t], in_=sa[:])
                nc.sync.dma_start(out=stats_p_d[t], in_=sp[:])
                nc.sync.dma_start(out=cvals_d[t], in_=cv[:])
                nc.sync.dma_start(out=cidx_d[t], in_=ci[:])

    if not nc.is_finalized():
        nc.finalize()
    _NC = nc
    return nc


def _run_device(student, teacher, trace=False, **kw):
    nc = _build_bass()
    in_maps = []
    for c in range(NCORES):
        r0 = c * RPC
        in_maps.append({
            "student": np.ascontiguousarray(student[r0:r0 + RPC]),
            "teacher": np.ascontiguousarray(teacher[r0:r0 + RPC]),
        })
    bkr = run_bass_kernel_spmd(nc, in_maps, core_ids=list(range(NCORES)),
                               trace=trace, **kw)
    return bkr


def _adw(i, j):
    t, tp = i + 1, j + 1
    return 1.0 / (1.5 + abs(t - tp)) * 2.0 * float(np.exp(-GAMMA * (t + tp)))


def _finalize(student, teacher, target, results):
    """Host epilogue in float64: O(B*K) work."""
    zce = np.empty((B,), np.float64)
    zs4 = np.empty((B,), np.float64)
    zt4 = np.empty((B,), np.float64)
    g = np.empty((B,), np.float64)
    sv = np.empty((B, K), np.float64)   # top-3 student values
    si = np.empty((B, K), np.int64)     # their vocab indices

    for c in range(NCORES):
        out = results[c]
        sa = out["stats_act"].reshape(RPC, 3 * NCH).astype(np.float64)
        sp = out["stats_g"].reshape(RPC, NCH).astype(np.float64)
        cval = out["cand_vals"].reshape(RPC, 8 * NCH)
        cidx = out["cand_idx"].reshape(RPC, 8 * NCH).astype(np.int64)
        r = slice(c * RPC, (c + 1) * RPC)
        zce[r] = sa[:, 0:NCH].sum(1)
        zs4[r] = sa[:, NCH:2 * NCH].sum(1)
        zt4[r] = sa[:, 2 * NCH:3 * NCH].sum(1)
        g[r] = sp.sum(1)
        # global vocab index of candidate j = local_idx + (j // 8) * W
        base = (np.arange(8 * NCH) // 8) * W
        gidx = cidx + base[None, :]
        order = np.argsort(-cval, axis=1, kind="stable")[:, :K]
        sv[r] = np.take_along_axis(cval, order, axis=1).astype(np.float64)
        si[r] = np.take_along_axis(gidx, order, axis=1)

    tgt = np.asarray(target).astype(np.int64).reshape(B)
    s_t = np.take_along_axis(student, tgt[:, None], axis=1)[:, 0].astype(np.float64)
    tv = np.take_along_axis(teacher, si, axis=1).astype(np.float64)  # teacher at top-3

    # CE (mean reduction)
    loss_ce = float(np.mean(np.log(zce) - s_t))

    # combo KLs over restricted softmaxes
    def restricted_kl(cols):
        a = tv[:, cols] / TEMP
        bq = sv[:, cols] / TEMP
        lse_a = np.log(np.sum(np.exp(a), axis=1, keepdims=True))
        lse_b = np.log(np.sum(np.exp(bq), axis=1, keepdims=True))
        lp = a - lse_a
        lq = bq - lse_b
        p = np.exp(lp)
        return np.sum(p * (lp - lq))  # sum over rows and entries

    combos = [(0, 1), (0, 2), (1, 2), (0, 1, 2)]
    total = 0.0
    for comb in combos:
        w = _adw(comb[0], comb[1]) if len(comb) == 2 else 1.0
        total += w * restricted_kl(list(comb)) * (TEMP ** 2) / B
    loss_kd = total / len(combos)

    # rNTK: complement-of-top3 KL via corrected full sums
    e_sv = np.exp(sv / TEMP)
    e_tv = np.exp(tv / TEMP)
    zsm = zs4 - e_sv.sum(1)
    ztm = zt4 - e_tv.sum(1)
    gm = g - np.sum(e_tv * (tv - sv), axis=1)
    kl_rntk = gm / (TEMP * ztm) - np.log(ztm) + np.log(zsm)
    not_loss_kd = float(np.sum(kl_rntk)) * (TEMP ** 2) / B

    return np.float32(loss_ce + loss_kd + not_loss_kd)


def kernel(logits_student, logits_teacher, target):
    student = np.ascontiguousarray(np.asarray(logits_student, dtype=np.float32))
    teacher = np.ascontiguousarray(np.asarray(logits_teacher, dtype=np.float32))
    bkr = _run_device(student, teacher, trace=False)
    return _finalize(student, teacher, target, bkr.results)



# revision 3
# speedup vs baseline: 1.0646x; 1.0646x over previous
"""Distillation loss (CE + top-k combo KLs + rNTK KL) on 8 Trainium2 cores.

v2a: bf16 streaming.  The reference's additive -1000 masks exactly restrict
each softmax to the unmasked entries, so the loss decomposes into per-row
scalars computable from single streaming passes:

  Zce = sum_v exp(s_v)          (CE logsumexp, temp 1)
  Zs4 = sum_v exp(s_v/4)        (student, temp 4)
  Zt4 = sum_v exp(t_v/4)        (teacher, temp 4)
  G   = sum_v exp(t_v/4)*(t_v - s_v)
  top-3 values + indices of s (per row)

Device (data-parallel over the batch, 256 rows/core): streams both logit
matrices once from HBM in bf16 (tolerance 2e-2 leaves plenty of precision
headroom), producing per-chunk partial sums + top-8-per-chunk candidates.
Host epilogue (O(B*K) work in float64): exact top-3 from candidates (with
dedup of bf16-tied indices), teacher/student gathers from the original fp32
tensors, the 3-term correction sums, the 4 tiny combo KLs, and the final
scalar.
"""

import sys

import numpy as np
import ml_dtypes

try:
    import concourse.bass as bass
except ImportError:  # pragma: no cover
    sys.path.insert(0, "/opt/trn_rl_repo")
    import concourse.bass as bass

import concourse.bacc as bacc
import concourse.mybir as mybir
from concourse.bass_utils import run_bass_kernel_spmd
from concourse.tile import TileContext

# Problem shape (hardcoded per spec).
B, V = 2048, 32000
NCORES = 8
RPC = B // NCORES          # rows per core = 256
P = 128                    # partitions
NT = RPC // P              # row tiles per core = 2
W = 4000                   # chunk width
NCH = V // W               # chunks per row tile = 8
K = 3
TEMP = 4.0
GAMMA = 0.05

F32 = mybir.dt.float32
BF16 = mybir.dt.bfloat16
U32 = mybir.dt.uint32
NP_BF16 = ml_dtypes.bfloat16

_NC = None


def _build_bass():
    global _NC
    if _NC is not None:
        return _NC

    nc = bacc.Bacc("TRN2", target_bir_lowering=False)

    s_d = nc.dram_tensor("student", [RPC, V], BF16, kind="ExternalInput")
    t_d = nc.dram_tensor("teacher", [RPC, V], BF16, kind="ExternalInput")
    # Per-chunk partials; host reduces. stats_act cols: [Zce | Zs4 | Zt4],
    # stats_g cols: [G] where G = sum(exp(t/4)*(t-s)).
    stats_a_d = nc.dram_tensor("stats_act", [NT, P, 3 * NCH], F32, kind="ExternalOutput")
    stats_p_d = nc.dram_tensor("stats_g", [NT, P, NCH], F32, kind="ExternalOutput")
    cvals_d = nc.dram_tensor("cand_vals", [NT, P, 8 * NCH], F32, kind="ExternalOutput")
    cidx_d = nc.dram_tensor("cand_idx", [NT, P, 8 * NCH], U32, kind="ExternalOutput")

    EXP = mybir.ActivationFunctionType.Exp
    MUL = mybir.AluOpType.mult
    SUB = mybir.AluOpType.subtract

    with TileContext(nc) as tc:
        with (
            tc.tile_pool(name="s", bufs=3) as s_pool,
            tc.tile_pool(name="t", bufs=3) as t_pool,
            tc.tile_pool(name="e", bufs=2) as e_pool,
            tc.tile_pool(name="d", bufs=2) as d_pool,
            tc.tile_pool(name="scr", bufs=1) as scr_pool,
            tc.tile_pool(name="small", bufs=2) as small_pool,
        ):
            # Write-only sink for the two student exps (ACT in-order; WAW only).
            scr_act = scr_pool.tile([P, W], BF16)

            for t in range(NT):
                sa = small_pool.tile([P, 3 * NCH], F32, tag="sa")
                sp = small_pool.tile([P, NCH], F32, tag="sp")
                cv = small_pool.tile([P, 8 * NCH], F32, tag="cv")
                ci = small_pool.tile([P, 8 * NCH], U32, tag="ci")
                r0 = t * P
                for c in range(NCH):
                    st = s_pool.tile([P, W], BF16)
                    tt = t_pool.tile([P, W], BF16)
                    et = e_pool.tile([P, W], BF16)
                    dt = d_pool.tile([P, W], BF16)
                    c0 = c * W
                    nc.sync.dma_start(out=st[:], in_=s_d[r0:r0 + P, c0:c0 + W])
                    nc.sync.dma_start(out=tt[:], in_=t_d[r0:r0 + P, c0:c0 + W])

                    # ACT: exp_t first so DVE's ttr unblocks early.
                    nc.scalar.activation(
                        out=et[:], in_=tt[:], func=EXP, scale=0.25,
                        accum_out=sa[:, 2 * NCH + c:2 * NCH + c + 1],
                    )
                    nc.scalar.activation(
                        out=scr_act[:], in_=st[:], func=EXP, scale=1.0,
                        accum_out=sa[:, c:c + 1],
                    )
                    nc.scalar.activation(
                        out=scr_act[:], in_=st[:], func=EXP, scale=0.25,
                        accum_out=sa[:, NCH + c:NCH + c + 1],
                    )

                    # Pool: diff = t - s (plain 2-input elementwise).
                    nc.gpsimd.tensor_tensor(out=dt[:], in0=tt[:], in1=st[:], op=SUB)

                    # DVE: per-chunk top-8 values + chunk-local indices,
                    # then fused multiply-reduce G_c = sum(diff * exp_t).
                    nc.vector.max(out=cv[:, c * 8:(c + 1) * 8], in_=st[:])
                    nc.vector.max_index(
                        out=ci[:, c * 8:(c + 1) * 8],
                        in_max=cv[:, c * 8:(c + 1) * 8],
                        in_values=st[:],
                    )
                    nc.vector.scalar_tensor_tensor(
                        out=dt[:], in0=dt[:], scalar=1.0, in1=et[:],
                        op0=MUL, op1=MUL,
                        accum_out=sp[:, c:c + 1],
                    )

                nc.sync.dma_start(out=stats_a_d[t], in_=sa[:])
                nc.sync.dma_start(out=stats_p_d[t], in_=sp[:])
                nc.sync.dma_start(out=cvals_d[t], in_=cv[:])
                nc.sync.dma_start(out=cidx_d[t], in_=ci[:])

    if not nc.is_finalized():
        nc.finalize()
    _NC = nc
    return nc


def _to_bf16(x):
    return np.asarray(x).astype(NP_BF16)


def _run_device(student, teacher, trace=False, **kw):
    """student/teacher: [B, V] arrays (converted to bf16 if needed)."""
    nc = _build_bass()
    if student.dtype != NP_BF16:
        student = _to_bf16(student)
    if teacher.dtype != NP_BF16:
        teacher = _to_bf16(teacher)
    in_maps = []
    for c in range(NCORES):
        r0 = c * RPC
        in_maps.append({
            "student": np.ascontiguousarray(student[r0:r0 + RPC]),
            "teacher": np.ascontiguousarray(teacher[r0:r0 + RPC]),
        })
    bkr = run_bass_kernel_spmd(nc, in_maps, core_ids=list(range(NCORES)),
                               trace=trace, **kw)
    return bkr


def _adw(i, j):
    t, tp = i + 1, j + 1
    return 1.0 / (1.5 + abs(t - tp)) * 2.0 * float(np.exp(-GAMMA * (t + tp)))


def _topk_distinct(cval, gidx, k):
    """Per-row top-k by value with distinct indices (bf16 ties can make
    max_index return the same index twice)."""
    B_, M = cval.shape
    order = np.argsort(-cval, axis=1, kind="stable")
    sv = np.empty((B_, k), np.float64)
    si = np.empty((B_, k), np.int64)
    oi = np.take_along_axis(gidx, order, axis=1)
    ov = np.take_along_axis(cval, order, axis=1)
    # fast path: first k are distinct for almost all rows
    fk_ok = np.ones(B_, dtype=bool)
    for a in range(k):
        for b in range(a + 1, k):
            fk_ok &= oi[:, a] != oi[:, b]
    sv[fk_ok] = ov[fk_ok, :k]
    si[fk_ok] = oi[fk_ok, :k]
    for r in np.nonzero(~fk_ok)[0]:
        seen = []
        vals = []
        for j in range(M):
            if oi[r, j] not in seen:
                seen.append(oi[r, j])
                vals.append(ov[r, j])
                if len(seen) == k:
                    break
        si[r] = seen
        sv[r] = vals
    return sv, si


def _finalize(student, teacher, target, results):
    """Host epilogue in float64: O(B*K) work."""
    zce = np.empty((B,), np.float64)
    zs4 = np.empty((B,), np.float64)
    zt4 = np.empty((B,), np.float64)
    g = np.empty((B,), np.float64)
    sv = np.empty((B, K), np.float64)   # top-3 student values
    si = np.empty((B, K), np.int64)     # their vocab indices

    for c in range(NCORES):
        out = results[c]
        sa = out["stats_act"].reshape(RPC, 3 * NCH).astype(np.float64)
        sp = out["stats_g"].reshape(RPC, NCH).astype(np.float64)
        cval = out["cand_vals"].reshape(RPC, 8 * NCH).astype(np.float64)
        cidx = out["cand_idx"].reshape(RPC, 8 * NCH).astype(np.int64)
        r = slice(c * RPC, (c + 1) * RPC)
        zce[r] = sa[:, 0:NCH].sum(1)
        zs4[r] = sa[:, NCH:2 * NCH].sum(1)
        zt4[r] = sa[:, 2 * NCH:3 * NCH].sum(1)
        g[r] = sp.sum(1)
        # global vocab index of candidate j = local_idx + (j // 8) * W
        base = (np.arange(8 * NCH) // 8) * W
        gidx = cidx + base[None, :]
        sv[r], si[r] = _topk_distinct(cval, gidx, K)

    tgt = np.asarray(target).astype(np.int64).reshape(B)
    s_t = np.take_along_axis(student, tgt[:, None], axis=1)[:, 0].astype(np.float64)
    # exact fp32 values at the selected indices (closer to reference than
    # the bf16 candidate values)
    sv = np.take_along_axis(student, si, axis=1).astype(np.float64)
    tv = np.take_along_axis(teacher, si, axis=1).astype(np.float64)

    # CE (mean reduction)
    loss_ce = float(np.mean(np.log(zce) - s_t))

    # combo KLs over restricted softmaxes
    def restricted_kl(cols):
        a = tv[:, cols] / TEMP
        bq = sv[:, cols] / TEMP
        lse_a = np.log(np.sum(np.exp(a), axis=1, keepdims=True))
        lse_b = np.log(np.sum(np.exp(bq), axis=1, keepdims=True))
        lp = a - lse_a
        lq = bq - lse_b
        p = np.exp(lp)
        return np.sum(p * (lp - lq))  # sum over rows and entries

    combos = [(0, 1), (0, 2), (1, 2), (0, 1, 2)]
    total = 0.0
    for comb in combos:
        w = _adw(comb[0], comb[1]) if len(comb) == 2 else 1.0
        total += w * restricted_kl(list(comb)) * (TEMP ** 2) / B
    loss_kd = total / len(combos)

    # rNTK: complement-of-top3 KL via corrected full sums
    e_sv = np.exp(sv / TEMP)
    e_tv = np.exp(tv / TEMP)
    zsm = zs4 - e_sv.sum(1)
    ztm = zt4 - e_tv.sum(1)
    gm = g - np.sum(e_tv * (tv - sv), axis=1)
    kl_rntk = gm / (TEMP * ztm) - np.log(ztm) + np.log(zsm)
    not_loss_kd = float(np.sum(kl_rntk)) * (TEMP ** 2) / B

    return np.float32(loss_ce + loss_kd + not_loss_kd)


def kernel(logits_student, logits_teacher, target):
    student = np.ascontiguousarray(np.asarray(logits_student, dtype=np.float32))
    teacher = np.ascontiguousarray(np.asarray(logits_teacher, dtype=np.float32))
    s16 = _to_bf16(student)
    t16 = _to_bf16(teacher)
    bkr = _run_device(s16, t16, trace=False)
    return _finalize(student, teacher, target, bkr.results)


# revision 6
# speedup vs baseline: 1.3029x; 1.2238x over previous
"""Distillation loss (CE + top-k combo KLs + rNTK KL) on 8 Trainium2 cores.

v2b: engine-balanced redesign.  The loss decomposes into per-row scalars:

  Zce = sum_v exp(s_v)       Zs4 = sum_v exp(s_v/4)     Zt4 = sum_v exp(t_v/4)
  Gt  = sum_v exp(t_v/4)*t_v Gs  = sum_v exp(t_v/4)*s_v (G = Gt - Gs)
  top-3 of s per row (values + indices)

Engine assignment per core (256 rows, data-parallel over batch):
  DMA   : srm bf16 row-major student (16.4MB) + ts fp8 transposed/interleaved
          [t|s|1] layout (16.4MB)  ~ 99us
  ACT   : exp(t/4) -> et (fp8), exp(s/4) -> es4 (bf16)  (2 passes, 110us)
  DVE   : max8 top-8-per-chunk candidates on srm + sq=es4^2 + part of q=sq^2
  Pool  : most of q = sq^2
  PE    : all vocab reductions.  Vocab lives on the partition axis (host
          pre-transposes), so matmuls with 128x128 stationary tiles compute
          diag(et^T t) = Gt, diag(et^T s) = Gs, a ones-column gives Zt4, and
          a ones-stationary over [es4|q] gives Zs4/Zce.  PSUM accumulates
          over all 250 vocab tiles.

Host epilogue (float64): exact top-3 recovered from candidate-flagged chunks
of the original fp32 student, teacher/student gathers, 3-term corrections,
4 tiny combo KLs, final scalar.
"""

import sys

import numpy as np
import ml_dtypes

try:
    import concourse.bass as bass
except ImportError:  # pragma: no cover
    sys.path.insert(0, "/opt/trn_rl_repo")
    import concourse.bass as bass

import concourse.bacc as bacc
import concourse.mybir as mybir
from concourse.bass_utils import run_bass_kernel_spmd
from concourse.tile import TileContext

# Problem shape (hardcoded per spec).
B, V = 2048, 32000
NCORES = 8
RPC = B // NCORES          # rows per core = 256
P = 128                    # partitions
NT = RPC // P              # row tiles per core = 2
W = 4000                   # row-major chunk width for max8
NCH = V // W               # chunks per row = 8
K = 3
TEMP = 4.0
GAMMA = 0.05

# transposed stream geometry
NVT = V // P               # vocab tiles = 250
CHT = 25                   # vocab tiles per chunk
NCHT = NVT // CHT          # transposed chunks = 10
TSW = 2 * 257              # ts cols per vocab tile: [t_h|s_h|1] x 2 halves
QSPL = 5                   # q tiles computed on DVE (rest on gpsimd)

F32 = mybir.dt.float32
BF16 = mybir.dt.bfloat16
FP8 = mybir.dt.float8e4
U32 = mybir.dt.uint32
NP_BF16 = ml_dtypes.bfloat16
NP_FP8 = ml_dtypes.float8_e4m3

_NC = None


def _build_bass():
    global _NC
    if _NC is not None:
        return _NC

    nc = bacc.Bacc("TRN2", target_bir_lowering=False)

    srm_d = nc.dram_tensor("srm", [RPC, V], BF16, kind="ExternalInput")
    ts_d = nc.dram_tensor("ts", [P, NVT * TSW], FP8, kind="ExternalInput")
    id_d = nc.dram_tensor("ident", [P, 129], BF16, kind="ExternalInput")
    zq_d = nc.dram_tensor("zq_out", [1, 512], F32, kind="ExternalOutput")
    gs_d = nc.dram_tensor("gstats", [P, 6], F32, kind="ExternalOutput")
    cv_d = nc.dram_tensor("cands", [NT, P, 8 * NCH], BF16, kind="ExternalOutput")

    EXP = mybir.ActivationFunctionType.Exp
    MUL = mybir.AluOpType.mult

    with TileContext(nc) as tc:
        with (
            tc.tile_pool(name="const", bufs=1) as const_pool,
            tc.tile_pool(name="ts", bufs=2) as ts_pool,
            tc.tile_pool(name="et", bufs=2) as et_pool,
            tc.tile_pool(name="esq", bufs=2) as esq_pool,
            tc.tile_pool(name="sq", bufs=2) as sq_pool,
            tc.tile_pool(name="srm", bufs=3) as srm_pool,
            tc.tile_pool(name="small", bufs=2) as small_pool,
            tc.psum_pool(name="ps", bufs=1) as ps_pool,
        ):
            ident = const_pool.tile([P, 129], BF16)
            nc.sync.dma_start(out=ident[:], in_=id_d[:, :])

            g_ps = [ps_pool.tile([P, 257], F32, tag=f"g{h}", name=f"g_ps{h}")
                    for h in range(2)]
            zq_ps = ps_pool.tile([1, 512], F32)

            cand_tiles = {}

            def emit_srm_chunk(k):
                rt, c = divmod(k, NCH)
                if c == 0:
                    cand_tiles[rt] = small_pool.tile([P, 8 * NCH], BF16,
                                                     tag="cand", name="cand_t")
                cand_t = cand_tiles[rt]
                srm_t = srm_pool.tile([P, W], BF16)
                r0, c0 = rt * P, c * W
                nc.sync.dma_start(out=srm_t[:],
                                  in_=srm_d[r0:r0 + P, c0:c0 + W])
                nc.vector.max(out=cand_t[:, c * 8:(c + 1) * 8], in_=srm_t[:])
                if c == NCH - 1:
                    nc.sync.dma_start(out=cv_d[rt], in_=cand_t[:])

            for ch in range(NCHT):
                ts_t = ts_pool.tile([P, CHT * TSW], FP8)
                nc.sync.dma_start(
                    out=ts_t[:], in_=ts_d[:, ch * CHT * TSW:(ch + 1) * CHT * TSW])
                ts_v = ts_t.rearrange("p (t h j) -> p t h j", t=CHT, h=2, j=257)

                et_t = et_pool.tile([P, CHT * 256], FP8)
                et_v = et_t.rearrange("p (t h j) -> p t h j", t=CHT, h=2, j=128)
                # esq: [p, {es4|q}, tile, half, col]
                esq_t = esq_pool.tile([P, 2, CHT, 2, 128], BF16)
                sq_t = sq_pool.tile([P, CHT, 2, 128], BF16)

                nc.scalar.activation(out=et_v, in_=ts_v[:, :, :, 0:128],
                                     func=EXP, scale=0.25)
                nc.scalar.activation(out=esq_t[:, 0], in_=ts_v[:, :, :, 128:256],
                                     func=EXP, scale=0.25)

                nc.vector.tensor_tensor(out=sq_t[:], in0=esq_t[:, 0],
                                        in1=esq_t[:, 0], op=MUL)
                nc.vector.tensor_tensor(out=esq_t[:, 1, 0:QSPL],
                                        in0=sq_t[:, 0:QSPL],
                                        in1=sq_t[:, 0:QSPL], op=MUL)
                nc.gpsimd.tensor_tensor(out=esq_t[:, 1, QSPL:],
                                        in0=sq_t[:, QSPL:],
                                        in1=sq_t[:, QSPL:], op=MUL)

                for v in range(CHT):
                    Vt = ch * CHT + v
                    st = (Vt == 0)
                    sp = (Vt == NVT - 1)
                    for h in range(2):
                        nc.tensor.matmul(
                            out=g_ps[h][:],
                            lhsT=et_t[:, v * 256 + h * 128:v * 256 + h * 128 + 128],
                            rhs=ts_v[:, v, h, :],
                            start=st, stop=sp)
                    nc.tensor.matmul(
                        out=zq_ps[:],
                        lhsT=ident[:, 128:129],
                        rhs=esq_t[:, :, v],
                        start=st, stop=sp)

                for k in range((ch * NT * NCH) // NCHT,
                               ((ch + 1) * NT * NCH) // NCHT):
                    emit_srm_chunk(k)

            # --- extraction ---
            gstat = small_pool.tile([P, 6], F32, tag="gstat")
            scrap = small_pool.tile([P, 128], BF16, tag="scrap")
            for h in range(2):
                nc.vector.scalar_tensor_tensor(
                    out=scrap[:], in0=g_ps[h][:, 0:128], scalar=1.0,
                    in1=ident[:, 0:128], op0=MUL, op1=MUL,
                    accum_out=gstat[:, 3 * h + 0:3 * h + 1])
                nc.vector.scalar_tensor_tensor(
                    out=scrap[:], in0=g_ps[h][:, 128:256], scalar=1.0,
                    in1=ident[:, 0:128], op0=MUL, op1=MUL,
                    accum_out=gstat[:, 3 * h + 1:3 * h + 2])
                nc.vector.tensor_copy(out=gstat[:, 3 * h + 2:3 * h + 3],
                                      in_=g_ps[h][:, 256:257])
            zq_sb = small_pool.tile([1, 512], F32, tag="zq")
            nc.vector.tensor_copy(out=zq_sb[:], in_=zq_ps[:])
            nc.sync.dma_start(out=gs_d[:, :], in_=gstat[:])
            nc.sync.dma_start(out=zq_d[:, :], in_=zq_sb[:])

    if not nc.is_finalized():
        nc.finalize()
    _NC = nc
    return nc


def _prep_core_inputs(student, teacher):
    """student/teacher: fp32 [B, V].  Returns per-core input maps."""
    s16 = student.astype(NP_BF16)
    s8 = student.astype(NP_FP8)
    t8 = teacher.astype(NP_FP8)

    ident = np.zeros((P, 129), dtype=NP_BF16)
    ident[np.arange(P), np.arange(P)] = 1.0
    ident[:, 128] = 1.0

    in_maps = []
    for c in range(NCORES):
        r0 = c * RPC
        # [v, p, h, j] = x[h*128+j, v*128+p]  (vocab tile v, partition p,
        # row-half h, row-in-half j)
        tt8 = np.ascontiguousarray(t8[r0:r0 + RPC]).T.reshape(NVT, P, 2, 128)
        ss8 = np.ascontiguousarray(s8[r0:r0 + RPC]).T.reshape(NVT, P, 2, 128)
        ts = np.empty((P, NVT, 2, 257), dtype=NP_FP8)
        ts[:, :, :, 0:128] = tt8.transpose(1, 0, 2, 3)
        ts[:, :, :, 128:256] = ss8.transpose(1, 0, 2, 3)
        ts[:, :, :, 256] = np.float32(1.0)
        in_maps.append({
            "srm": np.ascontiguousarray(s16[r0:r0 + RPC]),
            "ts": ts.reshape(P, NVT * TSW),
            "ident": ident,
        })
    return in_maps


def _run_device(student, teacher, trace=False, **kw):
    nc = _build_bass()
    student = np.asarray(student, dtype=np.float32)
    teacher = np.asarray(teacher, dtype=np.float32)
    in_maps = _prep_core_inputs(student, teacher)
    bkr = run_bass_kernel_spmd(nc, in_maps, core_ids=list(range(NCORES)),
                               trace=trace, **kw)
    return bkr


def _adw(i, j):
    t, tp = i + 1, j + 1
    return 1.0 / (1.5 + abs(t - tp)) * 2.0 * float(np.exp(-GAMMA * (t + tp)))


def _recover_top3(student, cands):
    """cands: [B, 64] candidate values (bf16, top-8 per 4000-chunk, desc).
    Returns exact fp32 top-3 values+indices per row, found by searching the
    original student data in the chunks flagged by the candidates."""
    cf = cands.astype(np.float32)
    m = cf[:, ::8]                                   # [B, 8] chunk tops
    th = np.partition(cf, -K, axis=1)[:, -K]         # 3rd largest candidate
    order = np.argsort(-m, axis=1, kind="stable")    # chunk ranking
    top4 = order[:, :4]                              # [B, 4]
    s3 = student.reshape(B, NCH, W)
    gath = np.take_along_axis(s3, top4[:, :, None], axis=1)  # [B, 4, W]
    flat = gath.reshape(B, 4 * W)
    idx3 = np.argpartition(-flat, K - 1, axis=1)[:, :K]
    vals = np.take_along_axis(flat, idx3, axis=1)
    vorder = np.argsort(-vals, axis=1, kind="stable")
    idx3 = np.take_along_axis(idx3, vorder, axis=1)
    vals = np.take_along_axis(vals, vorder, axis=1)
    gidx = np.take_along_axis(top4, idx3 // W, axis=1) * W + idx3 % W

    # rare fallback: a 5th chunk could still tie into the top-3 range
    m5 = np.take_along_axis(m, order[:, 4:5], axis=1)[:, 0]
    bad = np.nonzero(m5 >= th)[0]
    for r in bad:
        row = student[r]
        i3 = np.argpartition(-row, K - 1)[:K]
        i3 = i3[np.argsort(-row[i3], kind="stable")]
        gidx[r] = i3
        vals[r] = row[i3]
    return vals.astype(np.float64), gidx.astype(np.int64)


def _finalize(student, teacher, target, results):
    """Host epilogue in float64."""
    zce = np.empty((B,), np.float64)
    zs4 = np.empty((B,), np.float64)
    zt4 = np.empty((B,), np.float64)
    g = np.empty((B,), np.float64)
    cands = np.empty((B, 8 * NCH), NP_BF16)

    for c in range(NCORES):
        out = results[c]
        zq = out["zq_out"].reshape(512).astype(np.float64)
        gst = out["gstats"].reshape(P, 6).astype(np.float64)
        cands[c * RPC:(c + 1) * RPC] = out["cands"].reshape(RPC, 8 * NCH)
        for h in range(2):
            r = slice(c * RPC + h * P, c * RPC + (h + 1) * P)
            zs4[r] = zq[h * 128:(h + 1) * 128]
            zce[r] = zq[256 + h * 128:256 + (h + 1) * 128]
            g[r] = gst[:, 3 * h + 0] - gst[:, 3 * h + 1]
            zt4[r] = gst[:, 3 * h + 2]

    sv, si = _recover_top3(student, cands)

    tgt = np.asarray(target).astype(np.int64).reshape(B)
    s_t = np.take_along_axis(student, tgt[:, None], axis=1)[:, 0].astype(np.float64)
    tv = np.take_along_axis(teacher, si, axis=1).astype(np.float64)

    # CE (mean reduction)
    loss_ce = float(np.mean(np.log(zce) - s_t))

    # combo KLs over restricted softmaxes
    def restricted_kl(cols):
        a = tv[:, cols] / TEMP
        bq = sv[:, cols] / TEMP
        lse_a = np.log(np.sum(np.exp(a), axis=1, keepdims=True))
        lse_b = np.log(np.sum(np.exp(bq), axis=1, keepdims=True))
        lp = a - lse_a
        lq = bq - lse_b
        p = np.exp(lp)
        return np.sum(p * (lp - lq))  # sum over rows and entries

    combos = [(0, 1), (0, 2), (1, 2), (0, 1, 2)]
    total = 0.0
    for comb in combos:
        w = _adw(comb[0], comb[1]) if len(comb) == 2 else 1.0
        total += w * restricted_kl(list(comb)) * (TEMP ** 2) / B
    loss_kd = total / len(combos)

    # rNTK: complement-of-top3 KL via corrected full sums
    e_sv = np.exp(sv / TEMP)
    e_tv = np.exp(tv / TEMP)
    zsm = zs4 - e_sv.sum(1)
    ztm = zt4 - e_tv.sum(1)
    gm = g - np.sum(e_tv * (tv - sv), axis=1)
    kl_rntk = gm / (TEMP * ztm) - np.log(ztm) + np.log(zsm)
    not_loss_kd = float(np.sum(kl_rntk)) * (TEMP ** 2) / B

    return np.float32(loss_ce + loss_kd + not_loss_kd)


def kernel(logits_student, logits_teacher, target):
    student = np.ascontiguousarray(np.asarray(logits_student, dtype=np.float32))
    teacher = np.ascontiguousarray(np.asarray(logits_teacher, dtype=np.float32))
    bkr = _run_device(student, teacher, trace=False)
    return _finalize(student, teacher, target, bkr.results)


# revision 10
# speedup vs baseline: 1.4530x; 1.1153x over previous
"""Distillation loss (CE + top-k combo KLs + rNTK KL) on 8 Trainium2 cores.

v2b: engine-balanced redesign.  The loss decomposes into per-row scalars:

  Zce = sum_v exp(s_v)       Zs4 = sum_v exp(s_v/4)     Zt4 = sum_v exp(t_v/4)
  Gt  = sum_v exp(t_v/4)*t_v Gs  = sum_v exp(t_v/4)*s_v (G = Gt - Gs)
  top-3 of s per row (values + indices)

Engine assignment per core (256 rows, data-parallel over batch):
  DMA   : srm bf16 row-major student (16.4MB) + ts fp8 transposed/interleaved
          [t|s|1] layout (16.4MB)  ~ 99us
  ACT   : exp(t/4) -> et (fp8), exp(s/4) -> es4 (bf16)  (2 passes, 110us)
  DVE   : max8 top-8-per-chunk candidates on srm + sq=es4^2 + part of q=sq^2
  Pool  : most of q = sq^2
  PE    : all vocab reductions.  Vocab lives on the partition axis (host
          pre-transposes), so matmuls with 128x128 stationary tiles compute
          diag(et^T t) = Gt, diag(et^T s) = Gs, a ones-column gives Zt4, and
          a ones-stationary over [es4|q] gives Zs4/Zce.  PSUM accumulates
          over all 250 vocab tiles.

Host epilogue (float64): exact top-3 recovered from candidate-flagged chunks
of the original fp32 student, teacher/student gathers, 3-term corrections,
4 tiny combo KLs, final scalar.
"""

import sys

import numpy as np
import ml_dtypes

try:
    import concourse.bass as bass
except ImportError:  # pragma: no cover
    sys.path.insert(0, "/opt/trn_rl_repo")
    import concourse.bass as bass

import concourse.bacc as bacc
import concourse.mybir as mybir
from concourse.bass_utils import run_bass_kernel_spmd
from concourse.tile import TileContext

# Problem shape (hardcoded per spec).
B, V = 2048, 32000
NCORES = 8
RPC = B // NCORES          # rows per core = 256
P = 128                    # partitions
NT = RPC // P              # row tiles per core = 2
W = 4000                   # row-major chunk width for max8
NCH = V // W               # chunks per row = 8
K = 3
TEMP = 4.0
GAMMA = 0.05

# transposed stream geometry
NVT = V // P               # vocab tiles = 250
CHT = 10                   # vocab tiles per chunk
NCHT = NVT // CHT          # transposed chunks = 25
TSW = 2 * 257              # ts cols per vocab tile: [t_h|s_h|1] x 2 halves

F32 = mybir.dt.float32
BF16 = mybir.dt.bfloat16
FP8 = mybir.dt.float8e4
U32 = mybir.dt.uint32
NP_BF16 = ml_dtypes.bfloat16
NP_FP8 = ml_dtypes.float8_e4m3

_NC = None


def _build_bass():
    global _NC
    if _NC is not None:
        return _NC

    nc = bacc.Bacc("TRN2", target_bir_lowering=False)

    srm_d = nc.dram_tensor("srm", [RPC, V], BF16, kind="ExternalInput")
    ts_d = nc.dram_tensor("ts", [P, NVT * TSW], FP8, kind="ExternalInput")
    id_d = nc.dram_tensor("ident", [P, 129], BF16, kind="ExternalInput")
    zq_d = nc.dram_tensor("zq_out", [1, 512], F32, kind="ExternalOutput")
    gs_d = nc.dram_tensor("gstats", [P, 6], F32, kind="ExternalOutput")
    cv_d = nc.dram_tensor("cands", [NT, P, 8 * NCH], BF16, kind="ExternalOutput")

    EXP = mybir.ActivationFunctionType.Exp
    MUL = mybir.AluOpType.mult

    with TileContext(nc) as tc:
        with (
            tc.tile_pool(name="const", bufs=1) as const_pool,
            tc.tile_pool(name="ts", bufs=3) as ts_pool,
            tc.tile_pool(name="et", bufs=3) as et_pool,
            tc.tile_pool(name="esq", bufs=3) as esq_pool,
            tc.tile_pool(name="sq", bufs=3) as sq_pool,
            tc.tile_pool(name="srm", bufs=3) as srm_pool,
            tc.tile_pool(name="small", bufs=2) as small_pool,
            tc.psum_pool(name="ps", bufs=1) as ps_pool,
        ):
            ident = const_pool.tile([P, 129], BF16)
            nc.sync.dma_start(out=ident[:], in_=id_d[:, :])

            g_ps = [ps_pool.tile([P, 257], F32, tag=f"g{h}", name=f"g_ps{h}")
                    for h in range(2)]
            zs_ps = ps_pool.tile([1, 256], F32)
            zq2_ps = ps_pool.tile([1, 256], F32)

            cand_tiles = {}

            def emit_srm_chunk(k):
                rt, c = divmod(k, NCH)
                if c == 0:
                    cand_tiles[rt] = small_pool.tile([P, 8 * NCH], BF16,
                                                     tag="cand", name="cand_t")
                cand_t = cand_tiles[rt]
                srm_t = srm_pool.tile([P, W], BF16)
                r0, c0 = rt * P, c * W
                nc.sync.dma_start(out=srm_t[:],
                                  in_=srm_d[r0:r0 + P, c0:c0 + W])
                nc.vector.max(out=cand_t[:, c * 8:(c + 1) * 8], in_=srm_t[:])
                if c == NCH - 1:
                    nc.sync.dma_start(out=cv_d[rt], in_=cand_t[:])

            pending_zq = []

            def flush_zq():
                for esq_prev, v, st, sp in pending_zq:
                    nc.tensor.matmul(out=zq2_ps[:], lhsT=ident[:, 128:129],
                                     rhs=esq_prev[:, 1, v], start=st, stop=sp)
                pending_zq.clear()

            for ch in range(NCHT):
                ts_t = ts_pool.tile([P, CHT * TSW], FP8)
                nc.sync.dma_start(
                    out=ts_t[:], in_=ts_d[:, ch * CHT * TSW:(ch + 1) * CHT * TSW])
                ts_v = ts_t.rearrange("p (t h j) -> p t h j", t=CHT, h=2, j=257)

                et_t = et_pool.tile([P, CHT * 256], FP8)
                et_v = et_t.rearrange("p (t h j) -> p t h j", t=CHT, h=2, j=128)
                # esq: [p, {es4|q}, tile, half, col]
                esq_t = esq_pool.tile([P, 2, CHT, 2, 128], BF16)
                sq_t = sq_pool.tile([P, CHT, 2, 128], BF16)

                nc.scalar.activation(out=et_v, in_=ts_v[:, :, :, 0:128],
                                     func=EXP, scale=0.25)
                nc.scalar.activation(out=esq_t[:, 0], in_=ts_v[:, :, :, 128:256],
                                     func=EXP, scale=0.25)

                nc.vector.tensor_tensor(out=sq_t[:], in0=esq_t[:, 0],
                                        in1=esq_t[:, 0], op=MUL)
                nc.gpsimd.tensor_tensor(out=esq_t[:, 1], in0=sq_t[:],
                                        in1=sq_t[:], op=MUL)

                # q-dependent Zce matmuls run one chunk late so the PE never
                # waits on the gpsimd square chain
                flush_zq()
                for v in range(CHT):
                    Vt = ch * CHT + v
                    st = (Vt == 0)
                    sp = (Vt == NVT - 1)
                    for h in range(2):
                        nc.tensor.matmul(
                            out=g_ps[h][:],
                            lhsT=et_t[:, v * 256 + h * 128:v * 256 + h * 128 + 128],
                            rhs=ts_v[:, v, h, :],
                            start=st, stop=sp)
                    nc.tensor.matmul(
                        out=zs_ps[:], lhsT=ident[:, 128:129],
                        rhs=esq_t[:, 0, v], start=st, stop=sp)
                    pending_zq.append((esq_t, v, st, sp))

                for k in range((ch * NT * NCH) // NCHT,
                               ((ch + 1) * NT * NCH) // NCHT):
                    emit_srm_chunk(k)
            flush_zq()

            # --- extraction ---
            gstat = small_pool.tile([P, 6], F32, tag="gstat")
            scrap = small_pool.tile([P, 128], BF16, tag="scrap")
            for h in range(2):
                nc.vector.scalar_tensor_tensor(
                    out=scrap[:], in0=g_ps[h][:, 0:128], scalar=1.0,
                    in1=ident[:, 0:128], op0=MUL, op1=MUL,
                    accum_out=gstat[:, 3 * h + 0:3 * h + 1])
                nc.vector.scalar_tensor_tensor(
                    out=scrap[:], in0=g_ps[h][:, 128:256], scalar=1.0,
                    in1=ident[:, 0:128], op0=MUL, op1=MUL,
                    accum_out=gstat[:, 3 * h + 1:3 * h + 2])
                nc.vector.tensor_copy(out=gstat[:, 3 * h + 2:3 * h + 3],
                                      in_=g_ps[h][:, 256:257])
            zq_sb = small_pool.tile([1, 512], F32, tag="zq")
            nc.vector.tensor_copy(out=zq_sb[:, 0:256], in_=zs_ps[:])
            nc.vector.tensor_copy(out=zq_sb[:, 256:512], in_=zq2_ps[:])
            nc.sync.dma_start(out=gs_d[:, :], in_=gstat[:])
            nc.sync.dma_start(out=zq_d[:, :], in_=zq_sb[:])

    if not nc.is_finalized():
        nc.finalize()
    _NC = nc
    return nc


def _prep_core_inputs(student, teacher):
    """student/teacher: fp32 [B, V].  Returns per-core input maps."""
    s16 = student.astype(NP_BF16)
    s8 = student.astype(NP_FP8)
    t8 = teacher.astype(NP_FP8)

    ident = np.zeros((P, 129), dtype=NP_BF16)
    ident[np.arange(P), np.arange(P)] = 1.0
    ident[:, 128] = 1.0

    in_maps = []
    for c in range(NCORES):
        r0 = c * RPC
        # [v, p, h, j] = x[h*128+j, v*128+p]  (vocab tile v, partition p,
        # row-half h, row-in-half j)
        tt8 = np.ascontiguousarray(t8[r0:r0 + RPC]).T.reshape(NVT, P, 2, 128)
        ss8 = np.ascontiguousarray(s8[r0:r0 + RPC]).T.reshape(NVT, P, 2, 128)
        ts = np.empty((P, NVT, 2, 257), dtype=NP_FP8)
        ts[:, :, :, 0:128] = tt8.transpose(1, 0, 2, 3)
        ts[:, :, :, 128:256] = ss8.transpose(1, 0, 2, 3)
        ts[:, :, :, 256] = np.float32(1.0)
        in_maps.append({
            "srm": np.ascontiguousarray(s16[r0:r0 + RPC]),
            "ts": ts.reshape(P, NVT * TSW),
            "ident": ident,
        })
    return in_maps


def _run_device(student, teacher, trace=False, **kw):
    nc = _build_bass()
    student = np.asarray(student, dtype=np.float32)
    teacher = np.asarray(teacher, dtype=np.float32)
    in_maps = _prep_core_inputs(student, teacher)
    bkr = run_bass_kernel_spmd(nc, in_maps, core_ids=list(range(NCORES)),
                               trace=trace, **kw)
    return bkr


def _adw(i, j):
    t, tp = i + 1, j + 1
    return 1.0 / (1.5 + abs(t - tp)) * 2.0 * float(np.exp(-GAMMA * (t + tp)))


def _recover_top3(student, cands):
    """cands: [B, 64] candidate values (bf16, top-8 per 4000-chunk, desc).
    Returns exact fp32 top-3 values+indices per row, found by searching the
    original student data in the chunks flagged by the candidates."""
    cf = cands.astype(np.float32)
    m = cf[:, ::8]                                   # [B, 8] chunk tops
    th = np.partition(cf, -K, axis=1)[:, -K]         # 3rd largest candidate
    order = np.argsort(-m, axis=1, kind="stable")    # chunk ranking
    top4 = order[:, :4]                              # [B, 4]
    s3 = student.reshape(B, NCH, W)
    gath = np.take_along_axis(s3, top4[:, :, None], axis=1)  # [B, 4, W]
    flat = gath.reshape(B, 4 * W)
    idx3 = np.argpartition(-flat, K - 1, axis=1)[:, :K]
    vals = np.take_along_axis(flat, idx3, axis=1)
    vorder = np.argsort(-vals, axis=1, kind="stable")
    idx3 = np.take_along_axis(idx3, vorder, axis=1)
    vals = np.take_along_axis(vals, vorder, axis=1)
    gidx = np.take_along_axis(top4, idx3 // W, axis=1) * W + idx3 % W

    # rare fallback: a 5th chunk could still tie into the top-3 range
    m5 = np.take_along_axis(m, order[:, 4:5], axis=1)[:, 0]
    bad = np.nonzero(m5 >= th)[0]
    for r in bad:
        row = student[r]
        i3 = np.argpartition(-row, K - 1)[:K]
        i3 = i3[np.argsort(-row[i3], kind="stable")]
        gidx[r] = i3
        vals[r] = row[i3]
    return vals.astype(np.float64), gidx.astype(np.int64)


def _finalize(student, teacher, target, results):
    """Host epilogue in float64."""
    zce = np.empty((B,), np.float64)
    zs4 = np.empty((B,), np.float64)
    zt4 = np.empty((B,), np.float64)
    g = np.empty((B,), np.float64)
    cands = np.empty((B, 8 * NCH), NP_BF16)

    for c in range(NCORES):
        out = results[c]
        zq = out["zq_out"].reshape(512).astype(np.float64)
        gst = out["gstats"].reshape(P, 6).astype(np.float64)
        cands[c * RPC:(c + 1) * RPC] = out["cands"].reshape(RPC, 8 * NCH)
        for h in range(2):
            r = slice(c * RPC + h * P, c * RPC + (h + 1) * P)
            zs4[r] = zq[h * 128:(h + 1) * 128]
            zce[r] = zq[256 + h * 128:256 + (h + 1) * 128]
            g[r] = gst[:, 3 * h + 0] - gst[:, 3 * h + 1]
            zt4[r] = gst[:, 3 * h + 2]

    sv, si = _recover_top3(student, cands)

    tgt = np.asarray(target).astype(np.int64).reshape(B)
    s_t = np.take_along_axis(student, tgt[:, None], axis=1)[:, 0].astype(np.float64)
    tv = np.take_along_axis(teacher, si, axis=1).astype(np.float64)

    # CE (mean reduction)
    loss_ce = float(np.mean(np.log(zce) - s_t))

    # combo KLs over restricted softmaxes
    def restricted_kl(cols):
        a = tv[:, cols] / TEMP
        bq = sv[:, cols] / TEMP
        lse_a = np.log(np.sum(np.exp(a), axis=1, keepdims=True))
        lse_b = np.log(np.sum(np.exp(bq), axis=1, keepdims=True))
        lp = a - lse_a
        lq = bq - lse_b
        p = np.exp(lp)
        return np.sum(p * (lp - lq))  # sum over rows and entries

    combos = [(0, 1), (0, 2), (1, 2), (0, 1, 2)]
    total = 0.0
    for comb in combos:
        w = _adw(comb[0], comb[1]) if len(comb) == 2 else 1.0
        total += w * restricted_kl(list(comb)) * (TEMP ** 2) / B
    loss_kd = total / len(combos)

    # rNTK: complement-of-top3 KL via corrected full sums
    e_sv = np.exp(sv / TEMP)
    e_tv = np.exp(tv / TEMP)
    zsm = zs4 - e_sv.sum(1)
    ztm = zt4 - e_tv.sum(1)
    gm = g - np.sum(e_tv * (tv - sv), axis=1)
    kl_rntk = gm / (TEMP * ztm) - np.log(ztm) + np.log(zsm)
    not_loss_kd = float(np.sum(kl_rntk)) * (TEMP ** 2) / B

    return np.float32(loss_ce + loss_kd + not_loss_kd)


def kernel(logits_student, logits_teacher, target):
    student = np.ascontiguousarray(np.asarray(logits_student, dtype=np.float32))
    teacher = np.ascontiguousarray(np.asarray(logits_teacher, dtype=np.float32))
    bkr = _run_device(student, teacher, trace=False)
    return _finalize(student, teacher, target, bkr.results)
